# revision 52
# baseline (speedup 1.0000x reference)
"""Fused single-head attention (QKV projection + softmax(QK^T)V) on 8 trn2 cores.

Problem (hardcoded): x [4, 4096, 768] f32, W_qkv [768, 2304] f32, b_qkv [2304] f32.
  qkv = x @ W_qkv + b_qkv ; q,k,v = split(qkv, 3)
  out = softmax(q k^T / sqrt(768)) v          -> [4, 4096, 768] f32

Sharding: batch (4) x key-halves (2) -> 8 cores. Each core receives only
its own 2048 rows of x (its keys; for the odd core of a pair those are the
batch's rows [2048,4096), sent as that core's rows [0,2048)). It projects
q/k/v for those rows, the pair exchanges q halves with a pairwise
AllGather (placed so both cores hold the full q in unrotated query
order), and each core computes PARTIAL attention sums over its keys:
  outT_partial [768, 4096] = sum_j exp(q k_j^T / sqrt(H)) v_j
  den_partial  [4096]      = sum_j exp(q k_j^T / sqrt(H))
The host combines the partials of each pair: (o0 + o1) / (d0 + d1).
No max-subtraction is needed: scores here are O(1), exp is safe, and both
partials use the same (absent) shift so the combine is exact softmax.

Precision (gate: rel err < 2e-2; measured 1.78e-2, fp16 baseline 3.5e-4):
  - projections run in fp16 (fp8 projection fails the gate: ~2.6e-2).
  - q/k are stored fp8e4 and the score matmuls run DoubleRow (2x PE rate,
    3 matmuls of 256-contraction per 512-query block). Cost ~1.3e-2.
  - PV: the first NF8=8 of 16 key-tiles use fp8 p/v as DoubleRow j-tile
    pairs (6 matmuls per pair instead of 12); the rest fp16. The mean
    component of the v-quantization error is cancelled host-side with a
    rank-1 correction outer(vres, den)/NK, where vres = colsum(v - fp8(v))
    is computed on-device (DVE residual accum + ones-matmul partition sum).

On-chip layout ("transposed flash attention"):
  - qkv computed in head-major layout qT/kT [H, n] via lhsT=W, rhs=xT; v in
    [n, H] via lhsT=xT, rhs=W (no on-chip transposes anywhere).
  - scores computed transposed: sT[j, i] = (kT j-tile).T @ qT -> PSUM,
    exp via ScalarE (scale folded in).
  - denominator: S = sum_jt p_jt accumulated on VectorE (fp32 -> fp16),
    host finishes the partition-dim sum.
  - outT[h, i] += (v j-tile).T @ p accumulated over j in PSUM.
  - PV matmuls run behind QK (software pipeline); PSUM evacuations of
    i-block N are deferred into i-block N+1 (held two j-tiles so the new
    block's exps enqueue on ScalarE ahead of the evacuation copies).
  - phase 1 runs q first and overlaps the AllGather under the k/v passes.
  - DMA: host-interleaved [128, CT|HT, *] layouts make every x r-block / W
    section / output block a single coalesced DMA (each dma_start costs
    ~600ns of issue time on its engine queue).
PSUM budget (8 banks): scores 2 + out accumulators 6.
"""

import math
from contextlib import ExitStack
from functools import lru_cache

import numpy as np

import concourse.bacc as bacc
import concourse.bass as bass
import concourse.tile as tile
from concourse import mybir
from concourse.bass_utils import run_bass_kernel_spmd

B, N, C = 4, 4096, 768
H = 768          # head dim (== C)
H3 = 3 * H
NCORES = 8
NK = N // 2      # keys per core
DT = mybir.dt.float16
F8 = mybir.dt.float8e4
F32 = mybir.dt.float32
SCALE = 1.0 / math.sqrt(H)
USE_DR = True
NF8 = 8          # j-tiles (of JT=16) whose PV runs in fp8 DoubleRow pairs

CT = C // 128    # 6 contraction tiles (c)
HT = H // 128    # 6 head tiles (h)
JT = NK // 128   # 16 key tiles (j) per core
RB = 8           # r-blocks of 512 over the 4096 rows
RBS = N // RB    # 512
KRB = RB // 2    # r-blocks that contain this core's keys (first 4)
IB = 8           # i-blocks of 512 over all 4096 queries
IBS = N // IB    # 512


def build_program():
    nc = bacc.Bacc(
        "TRN2",
        target_bir_lowering=False,
        debug=False,
        enable_asserts=False,
        num_devices=NCORES,
    )
    # x and W are host-interleaved to [128, CT, *] so each r-block / W
    # section is a single coalesced DMA (dma_start costs ~600ns of sync
    # engine time each; the baseline's ~100 phase-1 DMAs serialized there).
    # Each core receives only its own half of the rows (keys == own query
    # half); the other half of q arrives via a pairwise AllGather.
    xT_d = nc.dram_tensor("xT", [128, CT, NK], DT, kind="ExternalInput").ap()
    w_d = nc.dram_tensor("w", [128, CT, H3], DT, kind="ExternalInput").ap()
    bqk_d = nc.dram_tensor("bqk", [128, 2 * HT], F32, kind="ExternalInput").ap()
    bv_d = nc.dram_tensor("bv", [128, H], F32, kind="ExternalInput").ap()
    outT_d = nc.dram_tensor("outT", [128, HT, N], DT, kind="ExternalOutput").ap()
    # per-partition partial softmax denominators; host sums over axis 1
    den_d = nc.dram_tensor("den", [IB, 128, IBS], DT, kind="ExternalOutput").ap()
    # column-sum over this core's fp8 keys of (v_true - v_fp8); the host adds
    # the rank-1 correction den_i * vres_h / NK to the partial numerators
    vres_d = nc.dram_tensor("vres", [1, H], F32, kind="ExternalOutput").ap()

    with tile.TileContext(nc) as tc:
        with ExitStack() as ctx:
            persist = ctx.enter_context(tc.tile_pool(name="persist", bufs=1))

            # q/k in fp8e4, laid out [128, HT, n] so DoubleRow matmuls can
            # take h-tile pairs via [:, 2g:2g+2, ...] (contraction = 256).
            kT = persist.tile([128, HT, NK], F8, tag="kT", name="kT")
            qT = persist.tile([128, HT, N], F8, tag="qT", name="qT")
            # v for j-tiles < NF8: fp8, stored as j-tile pairs for DoubleRow
            # PV; the rest fp16.
            v8p = [persist.tile([128, 2, H], F8, tag=f"v8p{t}", name=f"v8p{t}")
                   for t in range(NF8 // 2)]
            vv = {t: persist.tile([128, H], DT, tag=f"v{t}", name=f"v{t}")
                  for t in range(NF8, JT)}
            racc = persist.tile([128, H], DT, tag="racc", name="racc")
            bqk = persist.tile([128, 2 * HT], F32, tag="bqk")
            bvb = persist.tile([128, H], F32, tag="bvb")
            dramp = ctx.enter_context(
                tc.tile_pool(name="dramp", bufs=1, space="DRAM"))
            q_inb = dramp.tile([128, HT, NK], F8, name="q_inb")
            q_outb = dramp.tile([2, 128, HT, NK], F8, name="q_outb")

            # ---- Phase 1: QKV projection ----
            with tc.tile_pool(name="wpool", bufs=1) as wpool, \
                 tc.tile_pool(name="xpool", bufs=4) as xpool, \
                 tc.tile_pool(name="pj", bufs=4, space="PSUM") as pj, \
                 tc.tile_pool(name="pv", bufs=2, space="PSUM") as pv:

                ws = wpool.tile([128, CT, H3], DT, tag="ws", name="ws")
                # own-half q staging; shipped through the pairwise AllGather
                qstage = wpool.tile([128, HT, NK], F8, tag="qstage",
                                    name="qstage")

                def load_xt(rb):
                    r0 = rb * RBS
                    t = xpool.tile([128, CT, RBS], DT, tag="xt", name=f"xt{rb}")
                    nc.sync.dma_start(out=t, in_=xT_d[:, :, r0:r0 + RBS])
                    return t

                # DMA issue order = need order: q runs first (its result
                # feeds the cross-core exchange), so q's W columns and the
                # first x block are granular (per-ct) so the first matmuls
                # start as soon as their chunk lands; the rest one coalesced
                # DMA per section (each dma_start costs ~600ns of
                # sync-engine issue time).
                xts = [None] * KRB
                xt0 = xpool.tile([128, CT, RBS], DT, tag="xt", name="xt0")
                for ct in range(CT):
                    nc.sync.dma_start(out=ws[:, ct, 0:H],
                                      in_=w_d[:, ct, 0:H])
                    nc.sync.dma_start(out=xt0[:, ct, :],
                                      in_=xT_d[:, ct, 0:RBS])
                xts[0] = xt0
                nc.sync.dma_start(out=bqk, in_=bqk_d)
                xt1 = xpool.tile([128, CT, RBS], DT, tag="xt", name="xt1")
                for ct in range(CT):
                    nc.sync.dma_start(out=xt1[:, ct, :],
                                      in_=xT_d[:, ct, RBS:2 * RBS])
                xts[1] = xt1
                for rb in range(2, KRB):
                    xts[rb] = load_xt(rb)
                nc.sync.dma_start(out=ws[:, :, H:2 * H], in_=w_d[:, :, H:2 * H])
                nc.sync.dma_start(out=ws[:, :, 2 * H:H3], in_=w_d[:, :, 2 * H:H3])
                nc.sync.dma_start(out=bvb, in_=bv_d)

                # PE warm-up: ~3.4us of junk matmuls (no DMA deps) so the
                # HAM clock-gate reaches full rate while the first x/W DMAs
                # are still in flight.
                warm_l = xpool.tile([128, 128], DT, tag="warml", name="warml")
                warm_r = xpool.tile([128, 512], DT, tag="warmr", name="warmr")
                nc.vector.memset(warm_l, 0.0)
                nc.vector.memset(warm_r, 0.0)
                nc.vector.memset(racc, 0.0)
                # single accumulation chain: no per-MM semaphore round-trips
                wp = pj.tile([128, RBS], F32, tag="pj", name="warm")
                for i in range(8):
                    nc.tensor.matmul(wp, warm_l, warm_r,
                                     start=(i == 0), stop=(i == 7))

                def proj_qk(rb, wofs, dst):
                    r0 = rb * RBS
                    xt = xts[rb]
                    for ht in range(HT):
                        ps = pj.tile([128, RBS], F32, tag="pj")
                        for ct in range(CT):
                            nc.tensor.matmul(
                                ps,
                                ws[:, ct,
                                   wofs + ht * 128: wofs + (ht + 1) * 128],
                                xt[:, ct, :],
                                start=(ct == 0), stop=(ct == CT - 1),
                            )
                        bcol = (0 if wofs == 0 else HT) + ht
                        nc.scalar.activation(
                            out=dst[:, ht, r0:r0 + RBS],
                            in_=ps,
                            func=mybir.ActivationFunctionType.Identity,
                            bias=bqk[:, bcol:bcol + 1],
                        )

                # pass 1: q for all r-blocks, feeding the AllGather ASAP
                for rb in range(KRB):
                    proj_qk(rb, 0, qstage)

                # pairwise AllGather of the q halves: both cores of a pair
                # end up with the full q in unrotated query order (the host
                # therefore does not rotate the odd cores' outputs back).
                # Runs on GpSimd/DMA concurrently with the k/v passes below.
                nc.gpsimd.dma_start(out=q_inb[:], in_=qstage)
                nc.gpsimd.collective_compute(
                    "AllGather",
                    mybir.AluOpType.bypass,
                    replica_groups=[[2 * i, 2 * i + 1]
                                    for i in range(NCORES // 2)],
                    ins=[q_inb.opt()],
                    outs=[q_outb.opt()],
                )
                nc.gpsimd.dma_start(out=qT[:, :, 0:NK], in_=q_outb[0])
                nc.gpsimd.dma_start(out=qT[:, :, NK:N], in_=q_outb[1])

                # pass 2: k and v
                for rb in range(KRB):
                    r0 = rb * RBS
                    xt = xts[rb]
                    proj_qk(rb, H, kT)

                    if True:
                        for j in range(RBS // 128):
                            jt = rb * (RBS // 128) + j
                            ps = pv.tile([128, H], F32, tag="pv")
                            for ct in range(CT):
                                xs = xt[:, ct, j * 128:(j + 1) * 128]
                                nc.tensor.matmul(
                                    ps[:, 0:512], xs, ws[:, ct, 2 * H: 2 * H + 512],
                                    start=(ct == 0), stop=(ct == CT - 1))
                                nc.tensor.matmul(
                                    ps[:, 512:H], xs, ws[:, ct, 2 * H + 512: 3 * H],
                                    start=(ct == 0), stop=(ct == CT - 1))
                            if jt < NF8:
                                # fp8 v + residual for the host-side rank-1
                                # correction: racc += (v_true - fp8(v_true))
                                vf = xpool.tile([128, H], DT, tag="vf",
                                                name=f"vf{jt}")
                                rt = xpool.tile([128, H], DT, tag="rt",
                                                name=f"rt{jt}")
                                v8s = v8p[jt // 2][:, jt % 2, :]
                                nc.vector.tensor_add(vf, ps, bvb)
                                nc.scalar.activation(
                                    out=v8s, in_=vf,
                                    func=mybir.ActivationFunctionType.Copy)
                                nc.vector.tensor_sub(rt, vf, v8s)
                                nc.vector.tensor_add(racc, racc, rt)
                            else:
                                nc.vector.tensor_add(vv[jt], ps, bvb)

                # partition-sum racc via ones-matmul -> vres [1, H]
                ones = xpool.tile([128, 1], DT, tag="ones", name="ones")
                nc.vector.memset(ones, 1.0)
                c_sb = xpool.tile([1, H], F32, tag="csb", name="csb")
                cps = pv.tile([128, H], F32, tag="pv", name="cps")
                nc.tensor.matmul(cps[0:1, 0:512], ones, racc[:, 0:512],
                                 start=True, stop=True)
                nc.tensor.matmul(cps[0:1, 512:H], ones, racc[:, 512:H],
                                 start=True, stop=True)
                nc.vector.tensor_copy(out=c_sb, in_=cps[0:1, :])
                nc.sync.dma_start(out=vres_d, in_=c_sb)

            # ---- Phase 2: attention (partial sums over this core's keys) ----
            with tc.tile_pool(name="ppool", bufs=1) as ppool, \
                 tc.tile_pool(name="opool", bufs=2) as opool, \
                 tc.tile_pool(name="spool", bufs=2) as spool, \
                 tc.tile_pool(name="ps_s", bufs=2, space="PSUM") as ps_s, \
                 tc.tile_pool(name="ps_o", bufs=6, space="PSUM") as ps_o:
                p8_t = [ppool.tile([128, 2, IBS], F8, tag=f"p8{t}",
                                   name=f"p8{t}") for t in range(NF8 // 2)]
                p_t = {t: ppool.tile([128, IBS], DT, tag=f"p{t}", name=f"p{t}")
                       for t in range(NF8, JT)}

                def p_ap(jt):
                    if jt < NF8:
                        return p8_t[jt // 2][:, jt % 2, :]
                    return p_t[jt]

                pending = []   # deferred work, flushed between PE groups

                def flush():
                    while pending:
                        pending.pop(0)()

                def emit_pv8(og, pr):
                    # one DoubleRow group covers the j-tile pair (2pr, 2pr+1)
                    def go():
                        for ht in range(HT):
                            nc.tensor.matmul(
                                og[ht],
                                v8p[pr][:, :, ht * 128:(ht + 1) * 128],
                                p8_t[pr],
                                start=(pr == 0), stop=False,
                                perf_mode=mybir.MatmulPerfMode.DoubleRow,
                            )
                    pending.append(go)

                def emit_pv(og, jt, i0):
                    def go():
                        for ht in range(HT):
                            nc.tensor.matmul(
                                og[ht],
                                vv[jt][:, ht * 128:(ht + 1) * 128],
                                p_t[jt],
                                start=(jt == 0 and NF8 == 0),
                                stop=(jt == JT - 1),
                            )
                    pending.append(go)

                def emit_den_and_evac(og, S16, ib, i0):
                    def go():
                        nc.sync.dma_start(out=den_d[ib], in_=S16)
                        ot = opool.tile([128, HT, IBS], DT, tag="ot",
                                        name=f"ot{i0}")
                        for ht in range(HT):
                            if ht % 2 == 0:
                                nc.vector.tensor_copy(out=ot[:, ht, :],
                                                      in_=og[ht])
                            else:
                                nc.scalar.activation(
                                    out=ot[:, ht, :], in_=og[ht],
                                    func=mybir.ActivationFunctionType.Copy)
                        nc.scalar.dma_start(out=outT_d[:, :, i0:i0 + IBS],
                                            in_=ot)
                    pending.append(go)

                for ib in range(IB):
                    i0 = ib * IBS
                    og = [ps_o.tile([128, IBS], F32, tag="o", name=f"o{ib}_{g}")
                          for g in range(HT)]
                    Sf = spool.tile([128, IBS], F32, tag="Sf", name=f"Sf{ib}")
                    for jt in range(JT):
                        sps = ps_s.tile([128, IBS], F32, tag="s")
                        # hold the previous i-block's PV/evacuation flush for
                        # two j-tiles so this block's first exps enqueue on
                        # ScalarE ahead of the evacuation copies
                        hold_flush = (jt < 2 and ib > 0)
                        if USE_DR:
                            for g in range(HT // 2):
                                nc.tensor.matmul(
                                    sps,
                                    kT[:, 2 * g:2 * g + 2,
                                       jt * 128:(jt + 1) * 128],
                                    qT[:, 2 * g:2 * g + 2, i0:i0 + IBS],
                                    start=(g == 0), stop=(g == HT // 2 - 1),
                                    perf_mode=mybir.MatmulPerfMode.DoubleRow,
                                )
                        else:
                            for ht in range(HT):
                                nc.tensor.matmul(
                                    sps,
                                    kT[:, ht, jt * 128:(jt + 1) * 128],
                                    qT[:, ht, i0:i0 + IBS],
                                    start=(ht == 0), stop=(ht == HT - 1),
                                )
                        if not hold_flush:
                            flush()
                        nc.scalar.activation(
                            out=p_ap(jt), in_=sps,
                            func=mybir.ActivationFunctionType.Exp,
                            scale=SCALE,
                        )
                        if jt == 0:
                            nc.vector.tensor_copy(out=Sf, in_=p_ap(jt))
                        else:
                            nc.vector.tensor_add(Sf, Sf, p_ap(jt))
                        # pair 0 is deferred one extra j-tile so the previous
                        # i-block's PSUM evacuation (DVE/ScalarE) finishes
                        # before its banks are re-accumulated
                        if jt == 2 and NF8 >= 2:
                            emit_pv8(og, 0)
                        if jt < NF8:
                            if jt % 2 == 1 and jt > 1:
                                emit_pv8(og, jt // 2)
                        elif not (ib == IB - 1 and jt == JT - 1):
                            emit_pv(og, jt, i0)
                    S16 = spool.tile([128, IBS], DT, tag="S16", name=f"S16{ib}")
                    nc.vector.tensor_copy(out=S16, in_=Sf)
                    if ib < IB - 1:
                        emit_den_and_evac(og, S16, ib, i0)
                    else:
                        # eager epilogue: interleave the final j-tile's PV
                        # matmuls with per-h-tile evacuation so the output
                        # DMAs start as early as possible.
                        def epilogue(og=og, S16=S16, ib=ib, i0=i0):
                            nc.sync.dma_start(out=den_d[ib], in_=S16)
                            ot = opool.tile([128, HT, IBS], DT, tag="ot",
                                            name=f"ot{i0}")
                            for ht in range(HT):
                                nc.tensor.matmul(
                                    og[ht],
                                    vv[JT - 1][:, ht * 128:(ht + 1) * 128],
                                    p_t[JT - 1],
                                    start=False, stop=True,
                                )
                                if ht % 2 == 0:
                                    nc.vector.tensor_copy(out=ot[:, ht, :],
                                                          in_=og[ht])
                                    dma = nc.sync.dma_start
                                else:
                                    nc.scalar.activation(
                                        out=ot[:, ht, :], in_=og[ht],
                                        func=mybir.ActivationFunctionType.Copy)
                                    dma = nc.scalar.dma_start
                                # per-h-tile DMAs: each ships as soon as its
                                # slice is evacuated, minimizing the drain
                                dma(out=outT_d[:, ht:ht + 1, i0:i0 + IBS],
                                    in_=ot[:, ht:ht + 1, :])
                        pending.append(epilogue)
                flush()
    nc.compile()
    return nc


@lru_cache(maxsize=1)
def _cached_program():
    return build_program()


def _prep_in_maps(x, W_qkv, b_qkv):
    x = np.asarray(x, dtype=np.float32)
    W_qkv = np.asarray(W_qkv, dtype=np.float32)
    b_qkv = np.asarray(b_qkv, dtype=np.float32)
    # interleave [C, H3] -> [128, CT, H3] so W sections are single DMAs
    w16 = np.ascontiguousarray(
        W_qkv.astype(np.float16).reshape(CT, 128, H3).transpose(1, 0, 2))
    bq = b_qkv[0:H].astype(np.float32).reshape(HT, 128).T    # [128, HT]
    bk = b_qkv[H:2 * H].astype(np.float32).reshape(HT, 128).T
    bqk = np.ascontiguousarray(np.concatenate([bq, bk], axis=1))  # [128, 2*HT]
    bv = np.ascontiguousarray(
        np.broadcast_to(b_qkv[2 * H:3 * H].astype(np.float32), (128, H)))

    in_maps = []
    for core in range(NCORES):
        b, kh = core // 2, core % 2
        # this core's rows: keys == own query half
        xb = x[b][kh * NK:(kh + 1) * NK]   # [NK, C] f32
        xT = np.ascontiguousarray(
            xb.T.astype(np.float16).reshape(CT, 128, NK).transpose(1, 0, 2))
        in_maps.append({"xT": xT, "w": w16, "bqk": bqk, "bv": bv})
    return in_maps


def _unT(o):
    # [128, HT, N] fp16 -> [H, N] fp32
    return o.astype(np.float32).transpose(1, 0, 2).reshape(H, N)


def _combine(results):
    out = np.empty((B, N, C), dtype=np.float32)
    for b in range(B):
        r0, r1 = results[2 * b], results[2 * b + 1]
        o0 = _unT(r0["outT"])                    # [H, N]
        d0 = r0["den"].astype(np.float32).sum(axis=1).reshape(N)
        o1 = _unT(r1["outT"])
        d1 = r1["den"].astype(np.float32).sum(axis=1).reshape(N)
        # rank-1 correction for the fp8-quantized v tiles:
        # o += outer(vres, den) / NK.  (Both cores' outputs are already in
        # unrotated query order thanks to the AllGather placement.)
        o0 = o0 + np.outer(r0["vres"].reshape(H).astype(np.float32), d0) / NK
        o1 = o1 + np.outer(r1["vres"].reshape(H).astype(np.float32), d1) / NK
        out[b] = ((o0 + o1) / (d0 + d1)).T
    return out


def kernel(x, W_qkv, b_qkv):
    nc = _cached_program()
    in_maps = _prep_in_maps(x, W_qkv, b_qkv)
    res = run_bass_kernel_spmd(nc, in_maps, core_ids=list(range(NCORES)))
    return _combine(res.results)



# revision 53
# speedup vs baseline: 1.0012x; 1.0012x over previous
"""Fused single-head attention (QKV projection + softmax(QK^T)V) on 8 trn2 cores.

Problem (hardcoded): x [4, 4096, 768] f32, W_qkv [768, 2304] f32, b_qkv [2304] f32.
  qkv = x @ W_qkv + b_qkv ; q,k,v = split(qkv, 3)
  out = softmax(q k^T / sqrt(768)) v          -> [4, 4096, 768] f32

Sharding: batch (4) x key-halves (2) -> 8 cores. Each core receives only
its own 2048 rows of x (its keys; for the odd core of a pair those are the
batch's rows [2048,4096), sent as that core's rows [0,2048)). It projects
q/k/v for those rows, the pair exchanges q halves with a pairwise
AllGather (placed so both cores hold the full q in unrotated query
order), and each core computes PARTIAL attention sums over its keys:
  outT_partial [768, 4096] = sum_j exp(q k_j^T / sqrt(H)) v_j
  den_partial  [4096]      = sum_j exp(q k_j^T / sqrt(H))
The host combines the partials of each pair: (o0 + o1) / (d0 + d1).
No max-subtraction is needed: scores here are O(1), exp is safe, and both
partials use the same (absent) shift so the combine is exact softmax.

Precision (gate: rel err < 2e-2; measured 1.78e-2, fp16 baseline 3.5e-4):
  - projections run in fp16 (fp8 projection fails the gate: ~2.6e-2).
  - q/k are stored fp8e4 and the score matmuls run DoubleRow (2x PE rate,
    3 matmuls of 256-contraction per 512-query block). Cost ~1.3e-2.
  - PV: the first NF8=8 of 16 key-tiles use fp8 p/v as DoubleRow j-tile
    pairs (6 matmuls per pair instead of 12); the rest fp16. The mean
    component of the v-quantization error is cancelled host-side with a
    rank-1 correction outer(vres, den)/NK, where vres = colsum(v - fp8(v))
    is computed on-device (DVE residual accum + ones-matmul partition sum).

On-chip layout ("transposed flash attention"):
  - qkv computed in head-major layout qT/kT [H, n] via lhsT=W, rhs=xT; v in
    [n, H] via lhsT=xT, rhs=W (no on-chip transposes anywhere).
  - scores computed transposed: sT[j, i] = (kT j-tile).T @ qT -> PSUM,
    exp via ScalarE (scale folded in).
  - denominator: S = sum_jt p_jt accumulated on VectorE (fp32 -> fp16),
    host finishes the partition-dim sum.
  - outT[h, i] += (v j-tile).T @ p accumulated over j in PSUM.
  - PV matmuls run behind QK (software pipeline); PSUM evacuations of
    i-block N are deferred into i-block N+1 (held two j-tiles so the new
    block's exps enqueue on ScalarE ahead of the evacuation copies).
  - phase 1 runs q first and overlaps the AllGather under the k/v passes.
  - DMA: host-interleaved [128, CT|HT, *] layouts make every x r-block / W
    section / output block a single coalesced DMA (each dma_start costs
    ~600ns of issue time on its engine queue).
PSUM budget (8 banks): scores 2 + out accumulators 6.
"""

import math
from contextlib import ExitStack
from functools import lru_cache

import numpy as np

import concourse.bacc as bacc
import concourse.bass as bass
import concourse.tile as tile
from concourse import mybir
from concourse.bass_utils import run_bass_kernel_spmd

B, N, C = 4, 4096, 768
H = 768          # head dim (== C)
H3 = 3 * H
NCORES = 8
NK = N // 2      # keys per core
DT = mybir.dt.float16
F8 = mybir.dt.float8e4
F32 = mybir.dt.float32
SCALE = 1.0 / math.sqrt(H)
USE_DR = True
NF8 = 8          # j-tiles (of JT=16) whose PV runs in fp8 DoubleRow pairs

CT = C // 128    # 6 contraction tiles (c)
HT = H // 128    # 6 head tiles (h)
JT = NK // 128   # 16 key tiles (j) per core
RB = 8           # r-blocks of 512 over the 4096 rows
RBS = N // RB    # 512
KRB = RB // 2    # r-blocks that contain this core's keys (first 4)
IB = 8           # i-blocks of 512 over all 4096 queries
IBS = N // IB    # 512


def build_program():
    nc = bacc.Bacc(
        "TRN2",
        target_bir_lowering=False,
        debug=False,
        enable_asserts=False,
        num_devices=NCORES,
    )
    # x and W are host-interleaved to [128, CT, *] so each r-block / W
    # section is a single coalesced DMA (dma_start costs ~600ns of sync
    # engine time each; the baseline's ~100 phase-1 DMAs serialized there).
    # Each core receives only its own half of the rows (keys == own query
    # half); the other half of q arrives via a pairwise AllGather.
    xT_d = nc.dram_tensor("xT", [128, CT, NK], DT, kind="ExternalInput").ap()
    w_d = nc.dram_tensor("w", [128, CT, H3], DT, kind="ExternalInput").ap()
    bqk_d = nc.dram_tensor("bqk", [128, 2 * HT], F32, kind="ExternalInput").ap()
    bv_d = nc.dram_tensor("bv", [128, H], F32, kind="ExternalInput").ap()
    outT_d = nc.dram_tensor("outT", [128, HT, N], DT, kind="ExternalOutput").ap()
    # per-partition partial softmax denominators; host sums over axis 1
    den_d = nc.dram_tensor("den", [IB, 128, IBS], DT, kind="ExternalOutput").ap()
    # column-sum over this core's fp8 keys of (v_true - v_fp8); the host adds
    # the rank-1 correction den_i * vres_h / NK to the partial numerators
    vres_d = nc.dram_tensor("vres", [1, H], F32, kind="ExternalOutput").ap()

    with tile.TileContext(nc) as tc:
        with ExitStack() as ctx:
            persist = ctx.enter_context(tc.tile_pool(name="persist", bufs=1))

            # q/k in fp8e4, laid out [128, HT, n] so DoubleRow matmuls can
            # take h-tile pairs via [:, 2g:2g+2, ...] (contraction = 256).
            kT = persist.tile([128, HT, NK], F8, tag="kT", name="kT")
            qT = persist.tile([128, HT, N], F8, tag="qT", name="qT")
            # v for j-tiles < NF8: fp8, stored as j-tile pairs for DoubleRow
            # PV; the rest fp16.
            v8p = [persist.tile([128, 2, H], F8, tag=f"v8p{t}", name=f"v8p{t}")
                   for t in range(NF8 // 2)]
            vv = {t: persist.tile([128, H], DT, tag=f"v{t}", name=f"v{t}")
                  for t in range(NF8, JT)}
            racc = persist.tile([128, H], DT, tag="racc", name="racc")
            bqk = persist.tile([128, 2 * HT], F32, tag="bqk")
            bvb = persist.tile([128, H], F32, tag="bvb")
            dramp = ctx.enter_context(
                tc.tile_pool(name="dramp", bufs=1, space="DRAM"))
            q_inb = dramp.tile([128, HT, NK], F8, name="q_inb")
            q_outb = dramp.tile([2, 128, HT, NK], F8, name="q_outb")

            # ---- Phase 1: QKV projection ----
            with tc.tile_pool(name="wpool", bufs=1) as wpool, \
                 tc.tile_pool(name="xpool", bufs=4) as xpool, \
                 tc.tile_pool(name="pj", bufs=4, space="PSUM") as pj, \
                 tc.tile_pool(name="pv", bufs=2, space="PSUM") as pv:

                ws = wpool.tile([128, CT, H3], DT, tag="ws", name="ws")
                # own-half q staging; shipped through the pairwise AllGather
                qstage = wpool.tile([128, HT, NK], F8, tag="qstage",
                                    name="qstage")

                def load_xt(rb):
                    r0 = rb * RBS
                    t = xpool.tile([128, CT, RBS], DT, tag="xt", name=f"xt{rb}")
                    nc.sync.dma_start(out=t, in_=xT_d[:, :, r0:r0 + RBS])
                    return t

                # DMA issue order = need order: q runs first (its result
                # feeds the cross-core exchange), so q's W columns and the
                # first x block are granular (per-ct) so the first matmuls
                # start as soon as their chunk lands; the rest one coalesced
                # DMA per section (each dma_start costs ~600ns of
                # sync-engine issue time).
                xts = [None] * KRB
                xt0 = xpool.tile([128, CT, RBS], DT, tag="xt", name="xt0")
                for ct in range(CT):
                    nc.sync.dma_start(out=ws[:, ct, 0:H],
                                      in_=w_d[:, ct, 0:H])
                    nc.sync.dma_start(out=xt0[:, ct, :],
                                      in_=xT_d[:, ct, 0:RBS])
                xts[0] = xt0
                nc.sync.dma_start(out=bqk, in_=bqk_d)
                xt1 = xpool.tile([128, CT, RBS], DT, tag="xt", name="xt1")
                for ct in range(CT):
                    nc.sync.dma_start(out=xt1[:, ct, :],
                                      in_=xT_d[:, ct, RBS:2 * RBS])
                xts[1] = xt1
                for rb in range(2, KRB):
                    xts[rb] = load_xt(rb)
                nc.sync.dma_start(out=ws[:, :, H:2 * H], in_=w_d[:, :, H:2 * H])
                nc.sync.dma_start(out=ws[:, :, 2 * H:H3], in_=w_d[:, :, 2 * H:H3])
                nc.sync.dma_start(out=bvb, in_=bv_d)

                # PE warm-up: ~3.4us of junk matmuls (no DMA deps) so the
                # HAM clock-gate reaches full rate while the first x/W DMAs
                # are still in flight.
                warm_l = xpool.tile([128, 128], DT, tag="warml", name="warml")
                warm_r = xpool.tile([128, 512], DT, tag="warmr", name="warmr")
                nc.vector.memset(warm_l, 0.0)
                nc.vector.memset(warm_r, 0.0)
                nc.vector.memset(racc, 0.0)
                # single accumulation chain: no per-MM semaphore round-trips
                wp = pj.tile([128, RBS], F32, tag="pj", name="warm")
                for i in range(8):
                    nc.tensor.matmul(wp, warm_l, warm_r,
                                     start=(i == 0), stop=(i == 7))

                def proj_qk(rb, wofs, dst):
                    r0 = rb * RBS
                    xt = xts[rb]
                    for ht in range(HT):
                        ps = pj.tile([128, RBS], F32, tag="pj")
                        for ct in range(CT):
                            nc.tensor.matmul(
                                ps,
                                ws[:, ct,
                                   wofs + ht * 128: wofs + (ht + 1) * 128],
                                xt[:, ct, :],
                                start=(ct == 0), stop=(ct == CT - 1),
                            )
                        bcol = (0 if wofs == 0 else HT) + ht
                        nc.scalar.activation(
                            out=dst[:, ht, r0:r0 + RBS],
                            in_=ps,
                            func=mybir.ActivationFunctionType.Identity,
                            bias=bqk[:, bcol:bcol + 1],
                        )

                # pass 1: q for all r-blocks, feeding the AllGather ASAP
                for rb in range(KRB):
                    proj_qk(rb, 0, qstage)

                # pairwise AllGather of the q halves: both cores of a pair
                # end up with the full q in unrotated query order (the host
                # therefore does not rotate the odd cores' outputs back).
                # Runs on GpSimd/DMA concurrently with the k/v passes below.
                nc.gpsimd.dma_start(out=q_inb[:], in_=qstage)
                nc.gpsimd.collective_compute(
                    "AllGather",
                    mybir.AluOpType.bypass,
                    replica_groups=[[2 * i, 2 * i + 1]
                                    for i in range(NCORES // 2)],
                    ins=[q_inb.opt()],
                    outs=[q_outb.opt()],
                )
                nc.gpsimd.dma_start(out=qT[:, :, 0:NK], in_=q_outb[0])
                nc.gpsimd.dma_start(out=qT[:, :, NK:N], in_=q_outb[1])

                # pass 2: k and v
                for rb in range(KRB):
                    r0 = rb * RBS
                    xt = xts[rb]
                    proj_qk(rb, H, kT)

                    if True:
                        for j in range(RBS // 128):
                            jt = rb * (RBS // 128) + j
                            ps = pv.tile([128, H], F32, tag="pv")
                            for ct in range(CT):
                                xs = xt[:, ct, j * 128:(j + 1) * 128]
                                nc.tensor.matmul(
                                    ps[:, 0:512], xs, ws[:, ct, 2 * H: 2 * H + 512],
                                    start=(ct == 0), stop=(ct == CT - 1))
                                nc.tensor.matmul(
                                    ps[:, 512:H], xs, ws[:, ct, 2 * H + 512: 3 * H],
                                    start=(ct == 0), stop=(ct == CT - 1))
                            if jt < NF8:
                                # fp8 v + residual for the host-side rank-1
                                # correction: racc += (v_true - fp8(v_true))
                                vf = xpool.tile([128, H], DT, tag="vf",
                                                name=f"vf{jt}")
                                rt = xpool.tile([128, H], DT, tag="rt",
                                                name=f"rt{jt}")
                                v8s = v8p[jt // 2][:, jt % 2, :]
                                nc.vector.tensor_add(vf, ps, bvb)
                                nc.scalar.activation(
                                    out=v8s, in_=vf,
                                    func=mybir.ActivationFunctionType.Copy)
                                nc.vector.tensor_sub(rt, vf, v8s)
                                nc.vector.tensor_add(racc, racc, rt)
                            else:
                                nc.vector.tensor_add(vv[jt], ps, bvb)

                # partition-sum racc via ones-matmul -> vres [1, H]
                ones = xpool.tile([128, 1], DT, tag="ones", name="ones")
                nc.vector.memset(ones, 1.0)
                c_sb = xpool.tile([1, H], F32, tag="csb", name="csb")
                cps = pv.tile([128, H], F32, tag="pv", name="cps")
                nc.tensor.matmul(cps[0:1, 0:512], ones, racc[:, 0:512],
                                 start=True, stop=True)
                nc.tensor.matmul(cps[0:1, 512:H], ones, racc[:, 512:H],
                                 start=True, stop=True)
                nc.vector.tensor_copy(out=c_sb, in_=cps[0:1, :])
                nc.sync.dma_start(out=vres_d, in_=c_sb)

            # ---- Phase 2: attention (partial sums over this core's keys) ----
            with tc.tile_pool(name="ppool", bufs=1) as ppool, \
                 tc.tile_pool(name="opool", bufs=2) as opool, \
                 tc.tile_pool(name="spool", bufs=2) as spool, \
                 tc.tile_pool(name="ps_s", bufs=2, space="PSUM") as ps_s, \
                 tc.tile_pool(name="ps_o", bufs=6, space="PSUM") as ps_o:
                p8_t = [ppool.tile([128, 2, IBS], F8, tag=f"p8{t}",
                                   name=f"p8{t}") for t in range(NF8 // 2)]
                p_t = {t: ppool.tile([128, IBS], DT, tag=f"p{t}", name=f"p{t}")
                       for t in range(NF8, JT)}

                def p_ap(jt):
                    if jt < NF8:
                        return p8_t[jt // 2][:, jt % 2, :]
                    return p_t[jt]

                pending = []   # deferred work, flushed between PE groups

                def flush():
                    while pending:
                        pending.pop(0)()

                def emit_pv8(og, pr):
                    # one DoubleRow group covers the j-tile pair (2pr, 2pr+1)
                    def go():
                        for ht in range(HT):
                            nc.tensor.matmul(
                                og[ht],
                                v8p[pr][:, :, ht * 128:(ht + 1) * 128],
                                p8_t[pr],
                                start=(pr == 0), stop=False,
                                perf_mode=mybir.MatmulPerfMode.DoubleRow,
                            )
                    pending.append(go)

                def emit_pv(og, jt, i0):
                    def go():
                        for ht in range(HT):
                            nc.tensor.matmul(
                                og[ht],
                                vv[jt][:, ht * 128:(ht + 1) * 128],
                                p_t[jt],
                                start=(jt == 0 and NF8 == 0),
                                stop=(jt == JT - 1),
                            )
                    pending.append(go)

                def emit_den_and_evac(og, S16, ib, i0):
                    def go():
                        nc.sync.dma_start(out=den_d[ib], in_=S16)
                        ot = opool.tile([128, HT, IBS], DT, tag="ot",
                                        name=f"ot{i0}")
                        for ht in range(HT):
                            if ht % 2 == 0:
                                nc.vector.tensor_copy(out=ot[:, ht, :],
                                                      in_=og[ht])
                            else:
                                nc.scalar.activation(
                                    out=ot[:, ht, :], in_=og[ht],
                                    func=mybir.ActivationFunctionType.Copy)
                        nc.scalar.dma_start(out=outT_d[:, :, i0:i0 + IBS],
                                            in_=ot)
                    pending.append(go)

                for ib in range(IB):
                    i0 = ib * IBS
                    og = [ps_o.tile([128, IBS], F32, tag="o", name=f"o{ib}_{g}")
                          for g in range(HT)]
                    Sf = spool.tile([128, IBS], F32, tag="Sf", name=f"Sf{ib}")
                    for jt in range(JT):
                        sps = ps_s.tile([128, IBS], F32, tag="s")
                        # hold the previous i-block's PV/evacuation flush for
                        # two j-tiles so this block's first exps enqueue on
                        # ScalarE ahead of the evacuation copies
                        hold_flush = (jt < 2 and ib > 0)
                        if USE_DR:
                            for g in range(HT // 2):
                                nc.tensor.matmul(
                                    sps,
                                    kT[:, 2 * g:2 * g + 2,
                                       jt * 128:(jt + 1) * 128],
                                    qT[:, 2 * g:2 * g + 2, i0:i0 + IBS],
                                    start=(g == 0), stop=(g == HT // 2 - 1),
                                    perf_mode=mybir.MatmulPerfMode.DoubleRow,
                                )
                        else:
                            for ht in range(HT):
                                nc.tensor.matmul(
                                    sps,
                                    kT[:, ht, jt * 128:(jt + 1) * 128],
                                    qT[:, ht, i0:i0 + IBS],
                                    start=(ht == 0), stop=(ht == HT - 1),
                                )
                        if not hold_flush:
                            flush()
                        nc.scalar.activation(
                            out=p_ap(jt), in_=sps,
                            func=mybir.ActivationFunctionType.Exp,
                            scale=SCALE,
                        )
                        if jt == 0:
                            nc.vector.tensor_copy(out=Sf, in_=p_ap(jt))
                        else:
                            nc.vector.tensor_add(Sf, Sf, p_ap(jt))
                        # pair 0 is deferred one extra j-tile so the previous
                        # i-block's PSUM evacuation (DVE/ScalarE) finishes
                        # before its banks are re-accumulated
                        if jt == 2 and NF8 >= 2:
                            emit_pv8(og, 0)
                        if jt < NF8:
                            if jt % 2 == 1 and jt > 1:
                                emit_pv8(og, jt // 2)
                        elif not (ib == IB - 1 and jt == JT - 1):
                            emit_pv(og, jt, i0)
                    S16 = spool.tile([128, IBS], DT, tag="S16", name=f"S16{ib}")
                    nc.vector.tensor_copy(out=S16, in_=Sf)
                    if ib < IB - 1:
                        emit_den_and_evac(og, S16, ib, i0)
                    else:
                        # eager epilogue: interleave the final j-tile's PV
                        # matmuls with per-h-tile evacuation so the output
                        # DMAs start as early as possible.
                        def epilogue(og=og, S16=S16, ib=ib, i0=i0):
                            nc.sync.dma_start(out=den_d[ib], in_=S16)
                            ot = opool.tile([128, HT, IBS], DT, tag="ot",
                                            name=f"ot{i0}")
                            for ht in range(HT):
                                nc.tensor.matmul(
                                    og[ht],
                                    vv[JT - 1][:, ht * 128:(ht + 1) * 128],
                                    p_t[JT - 1],
                                    start=False, stop=True,
                                )
                                if ht % 2 == 0:
                                    nc.vector.tensor_copy(out=ot[:, ht, :],
                                                          in_=og[ht])
                                else:
                                    nc.scalar.activation(
                                        out=ot[:, ht, :], in_=og[ht],
                                        func=mybir.ActivationFunctionType.Copy)
                                    # ship h-tile pairs as they complete:
                                    # balances ~600ns/DMA issue cost against
                                    # starting the drain early
                                    dma = (nc.sync.dma_start if ht % 4 == 1
                                           else nc.scalar.dma_start)
                                    dma(out=outT_d[:, ht - 1:ht + 1,
                                                   i0:i0 + IBS],
                                        in_=ot[:, ht - 1:ht + 1, :])
                        pending.append(epilogue)
                flush()
    nc.compile()
    return nc


@lru_cache(maxsize=1)
def _cached_program():
    return build_program()


def _prep_in_maps(x, W_qkv, b_qkv):
    x = np.asarray(x, dtype=np.float32)
    W_qkv = np.asarray(W_qkv, dtype=np.float32)
    b_qkv = np.asarray(b_qkv, dtype=np.float32)
    # interleave [C, H3] -> [128, CT, H3] so W sections are single DMAs
    w16 = np.ascontiguousarray(
        W_qkv.astype(np.float16).reshape(CT, 128, H3).transpose(1, 0, 2))
    bq = b_qkv[0:H].astype(np.float32).reshape(HT, 128).T    # [128, HT]
    bk = b_qkv[H:2 * H].astype(np.float32).reshape(HT, 128).T
    bqk = np.ascontiguousarray(np.concatenate([bq, bk], axis=1))  # [128, 2*HT]
    bv = np.ascontiguousarray(
        np.broadcast_to(b_qkv[2 * H:3 * H].astype(np.float32), (128, H)))

    in_maps = []
    for core in range(NCORES):
        b, kh = core // 2, core % 2
        # this core's rows: keys == own query half
        xb = x[b][kh * NK:(kh + 1) * NK]   # [NK, C] f32
        xT = np.ascontiguousarray(
            xb.T.astype(np.float16).reshape(CT, 128, NK).transpose(1, 0, 2))
        in_maps.append({"xT": xT, "w": w16, "bqk": bqk, "bv": bv})
    return in_maps


def _unT(o):
    # [128, HT, N] fp16 -> [H, N] fp32
    return o.astype(np.float32).transpose(1, 0, 2).reshape(H, N)


def _combine(results):
    out = np.empty((B, N, C), dtype=np.float32)
    for b in range(B):
        r0, r1 = results[2 * b], results[2 * b + 1]
        o0 = _unT(r0["outT"])                    # [H, N]
        d0 = r0["den"].astype(np.float32).sum(axis=1).reshape(N)
        o1 = _unT(r1["outT"])
        d1 = r1["den"].astype(np.float32).sum(axis=1).reshape(N)
        # rank-1 correction for the fp8-quantized v tiles:
        # o += outer(vres, den) / NK.  (Both cores' outputs are already in
        # unrotated query order thanks to the AllGather placement.)
        o0 = o0 + np.outer(r0["vres"].reshape(H).astype(np.float32), d0) / NK
        o1 = o1 + np.outer(r1["vres"].reshape(H).astype(np.float32), d1) / NK
        out[b] = ((o0 + o1) / (d0 + d1)).T
    return out


def kernel(x, W_qkv, b_qkv):
    nc = _cached_program()
    in_maps = _prep_in_maps(x, W_qkv, b_qkv)
    res = run_bass_kernel_spmd(nc, in_maps, core_ids=list(range(NCORES)))
    return _combine(res.results)

